# revision 49
# baseline (speedup 1.0000x reference)
"""Multi-head attention (B=2, S=2048, D=1024, H=16) on 8 TRN2 NeuronCores.

Sharding: core c handles batch b = c//4 and head-group g = c%4 (4 heads each).
Each core computes Q/K/V projections for its 4 heads, masked-softmax
attention (the full [4, S, S] attention-probability slice is an output), and
a partial output projection. The host sums the 4 partial out-projections per
batch and concatenates attention slices across head groups.

Two kernel variants: a causal one (host detects mask == tril) that skips
fully-masked score tiles and writes zeros for the strict upper triangle, and
a general-mask fallback. Matmuls run as float32r (full PE rate); the P@V
path uses fp16 probabilities transposed in-flight by the DMA xbar.
"""

import math
from contextlib import ExitStack

import numpy as np

import concourse.bass as bass
import concourse.tile as tile
from concourse import bacc, mybir, masks
from concourse import bass_utils

F32 = mybir.dt.float32
R32 = mybir.dt.float32r
F16 = mybir.dt.float16
I32 = mybir.dt.int32
AF = mybir.ActivationFunctionType
ALU = mybir.AluOpType

# Full-problem constants
B, S, D, H = 2, 2048, 1024, 16
DK = 64                    # head dim
G = 4                      # heads per core
DL = G * DK                # local projection width (256)
N_CORES = 8


def _r(ap):
    return ap.bitcast(R32)


def build_nc(s=S, d=D, num_devices=N_CORES, causal=True):
    """Build the per-core Bass module. s/d shrinkable for simulation."""
    sc = 1.0 / math.sqrt(DK)
    n_st = s // 128            # number of 128-row s-tiles
    n_dc = d // 128            # number of 128-wide d chunks
    n_grp = max(1, n_st // 4)  # sq groups (4 tiles / 512 queries each)
    tpg = n_st // n_grp        # tiles per group
    grp_w = 128 * tpg          # queries per group

    nc = bacc.Bacc("TRN2", target_bir_lowering=False, debug=False,
                   num_devices=num_devices)

    xq = nc.dram_tensor("xq", (s, d), F32, kind="ExternalInput").ap()
    xk = nc.dram_tensor("xk", (s, d), F32, kind="ExternalInput").ap()
    xv = nc.dram_tensor("xv", (s, d), F32, kind="ExternalInput").ap()
    if causal:
        # stacked diagonal 128x128 mask blocks, additive form: 0 or -8e9
        mask = nc.dram_tensor("mask_diag", (s, 128), F32, kind="ExternalInput").ap()
    else:
        # additive form: 0 or -8e9
        mask = nc.dram_tensor("mask", (s, s), F32, kind="ExternalInput").ap()
    wq_t = nc.dram_tensor("wq_t", (d, DL), F32, kind="ExternalInput").ap()
    wk_t = nc.dram_tensor("wk_t", (d, DL), F32, kind="ExternalInput").ap()
    wv_t = nc.dram_tensor("wv_t", (d, DL), F32, kind="ExternalInput").ap()
    wo_s = nc.dram_tensor("wo_s", (DL, d), F32, kind="ExternalInput").ap()
    bq = nc.dram_tensor("bq", (DL,), F32, kind="ExternalInput").ap()
    bk = nc.dram_tensor("bk", (DL,), F32, kind="ExternalInput").ap()
    bv = nc.dram_tensor("bv", (DL,), F32, kind="ExternalInput").ap()
    attn_o = nc.dram_tensor("attn_o", (G, s, s), F32, kind="ExternalOutput").ap()
    out_o = nc.dram_tensor("out_o", (s, d), F32, kind="ExternalOutput").ap()

    with tile.TileContext(nc) as tc, ExitStack() as top:
        # ---- persistent SBUF ----
        per = top.enter_context(tc.tile_pool(name="persist", bufs=1))
        ident = per.tile([128, 128], F32)
        masks.make_identity(nc, ident[:])
        ident16 = per.tile([128, 128], F16)
        masks.make_identity(nc, ident16[:])
        qh_sb = per.tile([128, 2, s], R32, tag="qh")    # qh^T: [dl%128, dl//128, s]
        kh_sb = per.tile([128, 2, s], R32, tag="kh")
        vh_sb = per.tile([128, n_st, DL], F16, tag="vh")  # vh: [sk%128, sk//128, dl]
        ctx_sb = per.tile([128, 2, s], R32, tag="ctx")  # ctx^T like qh^T
        wo_sb = per.tile([128, 2, d], R32, tag="wo")
        wo_st = per.tile([128, 2, d], F32, tag="wo_st")
        nc.sync.dma_start(wo_st[:], wo_s.rearrange("(c p) n -> p c n", p=128))
        nc.vector.tensor_copy(wo_sb[:], wo_st[:])

        # ---- phase B: transposes + projections ----
        with ExitStack() as ph:
            wpool = ph.enter_context(tc.tile_pool(name="wts", bufs=1))
            xpool = ph.enter_context(tc.tile_pool(name="xwork", bufs=2))
            ptr_ps = ph.enter_context(tc.tile_pool(name="tr_ps", bufs=4, space="PSUM"))
            proj_ps = ph.enter_context(tc.tile_pool(name="proj_ps", bufs=2, space="PSUM"))

            vt_sb = xpool.tile([128, 2, s], F32, tag="vt", bufs=1)
            w_sb = {}
            b_sb = {}
            for name, wdr, bdr in (("q", wq_t, bq), ("k", wk_t, bk), ("v", wv_t, bv)):
                w_st = wpool.tile([128, n_dc, DL], F32, tag="w_st",
                                  name=f"wst_{name}", bufs=2)
                nc.sync.dma_start(w_st[:],
                                  wdr.rearrange("(c p) m -> p c m", p=128))
                w_sb[name] = wpool.tile([128, n_dc, DL], R32, tag=f"w{name}",
                                        name=f"w{name}")
                nc.vector.tensor_copy(w_sb[name][:], w_st[:])
                b_sb[name] = wpool.tile([128, 2], F32, tag=f"b{name}",
                                        name=f"b{name}")
                nc.sync.dma_start(b_sb[name][:],
                                  bdr.rearrange("(mh p) -> p mh", p=128))


            for name, xdr, copy_eng in (("q", xq, "v"), ("k", xk, "v"),
                                        ("v", xv, "s")):
                dst = {"q": qh_sb, "k": kh_sb, "v": vt_sb}[name]
                for scnk in range(s // 512):
                    x_nat = xpool.tile([128, 4, d], F32, tag="xnat")
                    nc.sync.dma_start(
                        x_nat[:],
                        xdr[512 * scnk:512 * (scnk + 1), :]
                        .rearrange("(t p) d -> p t d", p=128))
                    x_t = xpool.tile([128, n_dc, 512], R32, tag="xt")
                    for c in range(n_dc):
                        pst = ptr_ps.tile([128, 512], F32)
                        for t in range(4):
                            nc.tensor.transpose(
                                pst[:, 128 * t:128 * (t + 1)],
                                x_nat[:, t, 128 * c:128 * (c + 1)], ident[:])
                        if copy_eng == "v":
                            nc.vector.tensor_copy(x_t[:, c, :], pst[:])
                        else:
                            nc.scalar.copy(x_t[:, c, :], pst[:])
                    for mh in range(2):
                        psp = proj_ps.tile([128, 512], F32)
                        for c in range(n_dc):
                            nc.tensor.matmul(
                                psp[:],
                                w_sb[name][:, c, 128 * mh:128 * (mh + 1)],
                                x_t[:, c, :],
                                start=(c == 0), stop=(c == n_dc - 1))
                        nc.vector.tensor_scalar_add(
                            dst[:, mh, 512 * scnk:512 * (scnk + 1)], psp[:],
                            b_sb[name][:, mh:mh + 1])


            # detranspose vh^T [dl, s] -> vh [s%128, s//128, dl] fp16
            for mh in range(2):
                for sg in range(n_st // 4):
                    pstv = ptr_ps.tile([128, 512], F32, name="pstv", tag="pst")
                    for j in range(4):
                        skc = 4 * sg + j
                        nc.tensor.transpose(
                            pstv[:, 128 * j:128 * (j + 1)],
                            vt_sb[:, mh, 128 * skc:128 * (skc + 1)], ident[:])
                    nc.scalar.copy(
                        vh_sb[:, 4 * sg:4 * (sg + 1), 128 * mh:128 * (mh + 1)],
                        pstv[:].rearrange("p (j k) -> p j k", j=4))

        # ---- phase C: attention ----
        with ExitStack() as ph:
            apool = ph.enter_context(tc.tile_pool(name="attn_work", bufs=1))
            s_ps = ph.enter_context(tc.tile_pool(name="s_ps", bufs=4, space="PSUM"))
            v_ps = ph.enter_context(tc.tile_pool(name="v_ps", bufs=2, space="PSUM"))
            t_ps = ph.enter_context(tc.tile_pool(name="t_ps", bufs=2, space="PSUM"))

            for grp in range(n_grp):
                ptr = [apool.tile([128, n_st, grp_w], F16, tag=f"pt{h}", bufs=1,
                                  name=f"ptr{h}")
                       for h in range(G)]
                for stl in range(tpg):
                    st = tpg * grp + stl
                    rows = slice(128 * st, 128 * (st + 1))
                    if causal:
                        width = 128 * (st + 1)
                        mask_d = apool.tile([128, 128], F32, tag="mask", bufs=2)
                        nc.sync.dma_start(mask_d[:], mask[rows, :])
                    else:
                        width = s
                        mask_t = apool.tile([128, s], F32, tag="mask", bufs=2)
                        nc.sync.dma_start(mask_t[:], mask[rows, :])
                    nch = (width + 511) // 512
                    rs4 = apool.tile([128, G, 8], F32, tag="rs4", bufs=2)
                    for h in range(G):
                        bp = 64 * (h % 2)
                        mh = h // 2
                        qh_ap = qh_sb[bp:bp + 64, mh, rows]
                        p_t = apool.tile([128, s], F32, tag="p", bufs=5 if causal else 3,
                                         name=f"p{h}")
                        ncol = 0
                        pss = s_ps.tile([128, 512], F32, tag="pss")
                        for ch in range(nch):
                            c0 = 512 * ch
                            c1 = min(512 * (ch + 1), width)
                            cw = c1 - c0
                            nc.tensor.matmul(
                                pss[:, :cw], qh_ap,
                                kh_sb[bp:bp + 64, mh, c0:c1],
                                start=True, stop=True)
                            if not causal:
                                nc.vector.tensor_tensor(
                                    out=pss[:, :cw], in0=pss[:, :cw],
                                    in1=mask_t[:, c0:c1], op=ALU.add)
                                nc.scalar.activation(
                                    p_t[:, c0:c1], pss[:, :cw], AF.Exp, scale=sc,
                                    accum_out=rs4[:, h, ncol:ncol + 1])
                                ncol += 1
                            else:
                                if c1 == width:
                                    # apply additive diag mask to the last
                                    # 128 columns of the scores in PSUM
                                    fv = width - 128 - c0
                                    nc.vector.tensor_tensor(
                                        out=pss[:, fv:fv + 128],
                                        in0=pss[:, fv:fv + 128],
                                        in1=mask_d[:], op=ALU.add)
                                nc.scalar.activation(
                                    p_t[:, c0:c1], pss[:, :cw], AF.Exp, scale=sc,
                                    accum_out=rs4[:, h, ncol:ncol + 1])
                                ncol += 1
                        inv1 = apool.tile([128, 1], F32, tag="inv1",
                                          bufs=8, name=f"inv1_{h}")
                        nc.vector.reduce_sum(inv1[:], rs4[:, h, :ncol],
                                             axis=mybir.AxisListType.X)
                        nc.vector.reciprocal(inv1[:], inv1[:])
                        pn_t = apool.tile([128, s], F16, tag="pn", bufs=5 if causal else 3,
                                          name=f"pn{h}")
                        nc.vector.tensor_scalar(
                            out=pn_t[:, :width], in0=p_t[:, :width],
                            scalar1=inv1[:], scalar2=None, op0=ALU.mult)
                        # upper triangle left unwritten: output buffers
                        # are pre-zeroed by the runtime
                        nc.gpsimd.dma_start(attn_o[h, rows, :width],
                                            pn_t[:, :width])
                        nsk = width // 128
                        for sg in range((nsk + 3) // 4):
                            cnt = min(4, nsk - 4 * sg)
                            pstp = t_ps.tile([128, 512], F16, tag="pstp",
                                             name="pstp")
                            for j in range(cnt):
                                skc = 4 * sg + j
                                nc.tensor.transpose(
                                    pstp[:, 128 * j:128 * (j + 1)],
                                    pn_t[:, 128 * skc:128 * (skc + 1)],
                                    ident16[:])
                            dst = ptr[h][:, 4 * sg:4 * sg + cnt,
                                         128 * stl:128 * (stl + 1)]
                            srcap = pstp[:, :128 * cnt].rearrange(
                                "p (j k) -> p j k", j=cnt)
                            nc.vector.tensor_copy(dst, srcap)
                # PV for this group: col-packed head pairs
                n_skc = tpg * (grp + 1) if causal else n_st
                for hp in range(2):
                    psv = v_ps.tile([128, grp_w], F32, tag="psv", bufs=1,
                                    name="psv")
                    for hh in range(2):
                        h = 2 * hp + hh
                        for skc in range(n_skc):
                            lo = 128 * max(0, skc - tpg * grp) if causal else 0
                            nc.tensor.matmul(
                                psv[64 * hh:64 * (hh + 1), lo:],
                                vh_sb[:, skc, 64 * h:64 * (h + 1)],
                                ptr[h][:, skc, lo:],
                                start=(skc == 0), stop=(skc == n_skc - 1))
                    nc.vector.tensor_copy(
                        ctx_sb[:, hp, grp_w * grp:grp_w * (grp + 1)], psv[:])
                # out projection for this group's rows
                for stl in range(tpg):
                    st = tpg * grp + stl
                    out_t = apool.tile([128, d], F32, tag="out", bufs=2)
                    for nh in range(d // 512):
                        pso = v_ps.tile([128, 512], F32, tag="pso", bufs=1,
                                        name="pso")
                        for c in range(2):
                            nc.tensor.matmul(
                                pso[:], ctx_sb[:, c, 128 * st:128 * (st + 1)],
                                wo_sb[:, c, 512 * nh:512 * (nh + 1)],
                                start=(c == 0), stop=(c == 1))
                        nc.vector.tensor_copy(out_t[:, 512 * nh:512 * (nh + 1)],
                                              pso[:])
                    nc.sync.dma_start(out_o[128 * st:128 * (st + 1), :], out_t[:])

    nc.compile()
    return nc


_NC_CACHE = {}


def _get_nc(causal):
    if causal not in _NC_CACHE:
        _NC_CACHE[causal] = build_nc(causal=causal)
    return _NC_CACHE[causal]


def make_in_maps(q, k, v, mask2d, causal, Wq, bq, Wk, bk, Wv, bv, Wo):
    q = np.asarray(q, dtype=np.float32)
    k = np.asarray(k, dtype=np.float32)
    v = np.asarray(v, dtype=np.float32)
    if causal:
        diag = np.empty((S, 128), np.float32)
        for st in range(S // 128):
            blk = mask2d[128 * st:128 * (st + 1), 128 * st:128 * (st + 1)]
            diag[128 * st:128 * (st + 1)] = (blk - 1).astype(np.float32) * 8e9
        mask_in = {"mask_diag": diag}
    else:
        mask_in = {"mask": (mask2d - 1).astype(np.float32) * 8e9}
    in_maps = []
    for c in range(N_CORES):
        b, g = divmod(c, 4)
        sl = slice(DL * g, DL * (g + 1))
        in_maps.append({
            "xq": np.ascontiguousarray(q[b]),
            "xk": np.ascontiguousarray(k[b]),
            "xv": np.ascontiguousarray(v[b]),
            **mask_in,
            "wq_t": np.ascontiguousarray(np.asarray(Wq, np.float32)[sl, :].T),
            "wk_t": np.ascontiguousarray(np.asarray(Wk, np.float32)[sl, :].T),
            "wv_t": np.ascontiguousarray(np.asarray(Wv, np.float32)[sl, :].T),
            "wo_s": np.ascontiguousarray(np.asarray(Wo, np.float32)[:, sl].T),
            "bq": np.ascontiguousarray(np.asarray(bq, np.float32)[sl]),
            "bk": np.ascontiguousarray(np.asarray(bk, np.float32)[sl]),
            "bv": np.ascontiguousarray(np.asarray(bv, np.float32)[sl]),
        })
    return in_maps


def kernel(q, k, v, mask, num_heads, Wq, bq, Wk, bk, Wv, bv, Wo, bo):
    assert int(num_heads) == H
    mask2d = np.ascontiguousarray(np.asarray(mask, dtype=np.int32).reshape(S, S))
    causal = bool(np.array_equal(mask2d, np.tril(np.ones((S, S), np.int32))))
    nc = _get_nc(causal)
    in_maps = make_in_maps(q, k, v, mask2d, causal, Wq, bq, Wk, bk, Wv, bv, Wo)
    res = bass_utils.run_bass_kernel_spmd(nc, in_maps, core_ids=list(range(N_CORES)))
    attn = np.empty((B, H, S, S), dtype=np.float32)
    out = np.zeros((B, S, D), dtype=np.float32)
    for c in range(N_CORES):
        b, g = divmod(c, 4)
        attn[b, G * g:G * (g + 1)] = res.results[c]["attn_o"]
        out[b] += res.results[c]["out_o"]
    out += np.asarray(bo, np.float32)[None, None, :]
    return out, attn
